# revision 13
# baseline (speedup 1.0000x reference)
# DenseGATConv on 8 Trainium2 NeuronCores (Bass/Tile, SPMD over destination rows).
#
# Math: h = x@W ; el/er = head-wise <h, att> ; e_ij = leaky(el_i + er_j) ;
#       alpha = softmax_j(mask(e)) ; out_i = sum_j alpha_ij h_j + bias.
# Key identity: exp(leaky(s)) = max(exp(s), exp(0.2 s)) since exp is monotone
# and leaky(s) = max(s, 0.2 s).  With s_ij = el_i + er_j both branches are
# rank-1 outer products: exp(s) = exp(el_i) exp(er_j).  The masked unnormalized
# attention is  pm[j,i] = adj[i,j] * max(al_i*ar_j, bl_i*br_j)  which needs no
# transcendentals on the [N,N,H] tensor — just two fused DVE ops + a max.
# The denominator rides along as a ones-column in the aggregation matmul.
#
# Sharding: destination rows i split across 8 cores (512 rows each); every core
# computes the full h (it needs all source nodes j anyway); params replicated.
import numpy as np

N, IN_C, HEADS, OUT_C = 4096, 256, 4, 64
HC = HEADS * OUT_C          # 256
NCORES = 8
NB = N // NCORES            # 512 destination rows per core
JT = N // 128               # 32 source-node tiles
IT = NB // 128              # 4 row subtiles per core
C65 = OUT_C + 1             # head slice + ones column

TRACE = False               # test.py flips this to collect HW exec time
LAST_RESULTS = {}           # exec_time_ns etc. stashed here when TRACE

_compiled = {}


def _emit(ctx, tc, nc, io):
    import concourse.bass as bass
    import concourse.masks as masks
    from concourse import mybir

    dt = mybir.dt
    Alu = mybir.AluOpType
    Act = mybir.ActivationFunctionType

    xT, xoT, adjbT, W, Wal, War, bias, out = (
        io["xT"], io["xoT"], io["adjbT"], io["W"], io["Wal"], io["War"],
        io["bias"], io["out"],
    )

    big = ctx.enter_context(tc.tile_pool(name="big", bufs=1))
    tr = ctx.enter_context(tc.tile_pool(name="tr", bufs=3))
    adjpool = ctx.enter_context(tc.tile_pool(name="adjpool", bufs=2))
    ps = ctx.enter_context(tc.tile_pool(name="ps", bufs=2, space="PSUM"))
    pacc = ctx.enter_context(tc.tile_pool(name="pacc", bufs=1, space="PSUM"))

    # ---- constants / params -------------------------------------------------
    idf = big.tile([128, 128], dt.float32, tag="idf")
    masks.make_identity(nc, idf[:])
    ones_row = big.tile([1, 128], dt.float32, tag="ones_row")
    nc.vector.memset(ones_row[:], 1.0)

    bias_b = big.tile([128, HC], dt.float32, tag="bias_b")
    bias_bcast_ap = bass.AP(
        tensor=bias.tensor, offset=bias.offset, ap=[[0, 128]] + list(bias.ap)
    )
    nc.gpsimd.dma_start(out=bias_b[:], in_=bias_bcast_ap)

    wfull = []
    wb = []
    wal = []
    war = []
    for ct in range(2):
        wf = big.tile([128, HC], dt.float32, tag=f"wf{ct}")
        nc.sync.dma_start(out=wf[:], in_=W[ct * 128:(ct + 1) * 128, :])
        wfull.append(wf)
        w16 = big.tile([128, HC], dt.bfloat16, tag=f"wb{ct}")
        nc.vector.tensor_copy(w16[:], wf[:])
        wb.append(w16)
        wl = big.tile([128, HEADS], dt.float32, tag=f"wal{ct}")
        nc.sync.dma_start(out=wl[:], in_=Wal[ct * 128:(ct + 1) * 128, :])
        wal.append(wl)
        wr = big.tile([128, HEADS], dt.float32, tag=f"war{ct}")
        nc.sync.dma_start(out=wr[:], in_=War[ct * 128:(ct + 1) * 128, :])
        war.append(wr)

    # ---- xT load + bf16 cast ------------------------------------------------
    xTf = []
    xTb = []
    for ct in range(2):
        xf = big.tile([128, N], dt.float32, tag=f"xTf{ct}")
        nc.sync.dma_start(out=xf[:], in_=xT[ct * 128:(ct + 1) * 128, :])
        xTf.append(xf)
        xb = big.tile([128, N], dt.bfloat16, tag=f"xTb{ct}")
        nc.vector.tensor_copy(xb[:], xf[:])
        xTb.append(xb)
    xo = []
    for ct in range(2):
        t = big.tile([128, NB], dt.float32, tag=f"xoT{ct}")
        nc.sync.dma_start(out=t[:], in_=xoT[ct * 128:(ct + 1) * 128, :])
        xo.append(t)

    # ---- h65 (bf16 h + ones col, per j-tile) and er (f32) -------------------
    h65 = []
    er_pack = big.tile([128, JT * HEADS], dt.float32, tag="er_pack")  # [128,128]
    for nt in range(JT):
        hps = ps.tile([128, HC], dt.float32, tag="scr")
        for ct in range(2):
            nc.tensor.matmul(
                hps[:], lhsT=xTb[ct][:, nt * 128:(nt + 1) * 128], rhs=wb[ct][:],
                start=(ct == 0), stop=(ct == 1),
            )
        ht = big.tile([128, HEADS * C65], dt.bfloat16, tag=f"h65_{nt}")
        hr = ht[:].rearrange("p (h c) -> p h c", c=C65)
        hpr = hps[:].rearrange("p (h c) -> p h c", c=OUT_C)
        nc.any.tensor_copy(hr[:, :, 0:OUT_C], hpr[:, :, :])
        nc.vector.memset(hr[:, :, OUT_C], 1.0)
        h65.append(ht)

        eps = ps.tile([128, HEADS], dt.float32, tag="scr")
        for ct in range(2):
            nc.tensor.matmul(
                eps[:], lhsT=xTf[ct][:, nt * 128:(nt + 1) * 128], rhs=war[ct][:],
                start=(ct == 0), stop=(ct == 1),
            )
        nc.any.tensor_copy(er_pack[:, nt * HEADS:(nt + 1) * HEADS], eps[:])

    # exp factors for the source side (per-partition scalars, natural layout)
    ar_pack = big.tile([128, JT * HEADS], dt.float32, tag="ar_pack")
    br_pack = big.tile([128, JT * HEADS], dt.float32, tag="br_pack")
    nc.scalar.activation(ar_pack[:], er_pack[:], Act.Exp)
    nc.scalar.activation(br_pack[:], er_pack[:], Act.Exp, scale=0.2)

    # ---- elT (destination side, transposed layout) + exp + broadcast --------
    al_b = []
    bl_b = []
    for h in range(HEADS):
        # one [1, NB] matmul per head so every el row sits at partition 0
        # (engine access patterns must start at a 32-aligned partition)
        elp = ps.tile([1, NB], dt.float32, tag="scr")
        for ct in range(2):
            nc.tensor.matmul(
                elp[:], lhsT=wal[ct][:, h:h + 1], rhs=xo[ct][:],
                start=(ct == 0), stop=(ct == 1),
            )
        for scale, lst in ((1.0, al_b), (0.2, bl_b)):
            row = big.tile([1, NB], dt.float32, tag=f"albl_{h}_{scale}")
            nc.scalar.activation(row[:], elp[:], Act.Exp, scale=scale)
            bps = ps.tile([128, NB], dt.float32, tag="scr")
            nc.tensor.matmul(bps[:], lhsT=ones_row[:], rhs=row[:],
                             start=True, stop=True)
            bt = big.tile([128, NB], dt.bfloat16, tag=f"albc_{h}_{scale}")
            nc.any.tensor_copy(bt[:], bps[:])
            lst.append(bt)

    # ---- adjacency: host-pretransposed [N, NB]; load + cast bf16 (0/1) ------
    adjT = []
    for jt in range(JT):
        ai = adjpool.tile([128, NB], dt.int32, tag="adjint")
        nc.sync.dma_start(out=ai[:], in_=adjbT[jt * 128:(jt + 1) * 128, :])
        ab = big.tile([128, NB], dt.bfloat16, tag=f"adjT{jt}", name=f"adjT{jt}")
        nc.vector.tensor_copy(ab[:], ai[:])
        adjT.append(ab)

    # ---- main loop: per source tile, per head -------------------------------
    # A = (al_i*ar_j), B = (bl_i*br_j) as 4x-mode tensor_scalar; max on DVE;
    # adjacency mask multiply on GpSimd (otherwise idle) to split the load.
    po = [pacc.tile([C65, NB], dt.float32, name=f"o{h}", tag=f"o{h}") for h in range(HEADS)]
    for jt in range(JT):
        for h in range(HEADS):
            col = jt * HEADS + h
            am = tr.tile([128, NB], dt.bfloat16, tag="am")
            nc.vector.tensor_scalar_mul(am[:], al_b[h][:], ar_pack[:, col:col + 1])
            bm = tr.tile([128, NB], dt.bfloat16, tag="bm")
            nc.vector.tensor_scalar_mul(bm[:], bl_b[h][:], br_pack[:, col:col + 1])
            q = tr.tile([128, NB], dt.bfloat16, tag="q")
            nc.vector.tensor_max(q[:], am[:], bm[:])
            pm = tr.tile([128, NB], dt.bfloat16, tag="pm")
            nc.gpsimd.tensor_mul(pm[:], q[:], adjT[jt][:])
            nc.tensor.matmul(
                po[h][:], lhsT=h65[jt][:, h * C65:(h + 1) * C65], rhs=pm[:],
                start=(jt == 0), stop=(jt == JT - 1),
            )

    # ---- epilogue: transpose back, normalize, bias, store -------------------
    osb = [tr.tile([C65, NB], dt.float32, name=f"osb{h}", tag=f"osb{h}") for h in range(HEADS)]
    for h in range(HEADS):
        nc.any.tensor_copy(osb[h][:], po[h][:])
    for it in range(IT):
        ot = tr.tile([128, HC], dt.float32, tag="ot")
        for h in range(HEADS):
            pt = ps.tile([128, C65], dt.float32, tag="scr")
            nc.tensor.transpose(
                pt[:], osb[h][:, it * 128:(it + 1) * 128], idf[0:C65, 0:C65]
            )
            rec = tr.tile([128, 1], dt.float32, tag="rec")
            nc.vector.reciprocal(rec[:], pt[:, OUT_C:OUT_C + 1])
            nc.vector.scalar_tensor_tensor(
                out=ot[:, h * OUT_C:(h + 1) * OUT_C], in0=pt[:, 0:OUT_C],
                scalar=rec[:], in1=bias_b[:, h * OUT_C:(h + 1) * OUT_C],
                op0=Alu.mult, op1=Alu.add,
            )
        nc.sync.dma_start(out=out[it * 128:(it + 1) * 128, :], in_=ot[:])


def build():
    from contextlib import ExitStack
    import concourse.bacc as bacc
    import concourse.tile as tile
    from concourse import mybir

    dt = mybir.dt
    nc = bacc.Bacc("TRN2", target_bir_lowering=False, debug=False,
                   num_devices=NCORES)
    io = {
        "xT": nc.dram_tensor("xT", [IN_C, N], dt.float32, kind="ExternalInput").ap(),
        "xoT": nc.dram_tensor("xoT", [IN_C, NB], dt.float32, kind="ExternalInput").ap(),
        "adjbT": nc.dram_tensor("adjbT", [N, NB], dt.int32, kind="ExternalInput").ap(),
        "W": nc.dram_tensor("W", [IN_C, HC], dt.float32, kind="ExternalInput").ap(),
        "Wal": nc.dram_tensor("Wal", [IN_C, HEADS], dt.float32, kind="ExternalInput").ap(),
        "War": nc.dram_tensor("War", [IN_C, HEADS], dt.float32, kind="ExternalInput").ap(),
        "bias": nc.dram_tensor("bias", [HC], dt.float32, kind="ExternalInput").ap(),
        "out": nc.dram_tensor("out", [NB, HC], dt.float32, kind="ExternalOutput").ap(),
    }
    with tile.TileContext(nc) as tc:
        with ExitStack() as ctx:
            _emit(ctx, tc, nc, io)
    nc.compile()
    return nc


def make_in_maps(x, adj, W, att_l, att_r, bias):
    x = np.asarray(x, np.float32)
    adj = np.ascontiguousarray(np.asarray(adj, np.int32))
    W = np.asarray(W, np.float32)
    att_l = np.asarray(att_l, np.float32)
    att_r = np.asarray(att_r, np.float32)
    bias = np.asarray(bias, np.float32)
    xT = np.ascontiguousarray(x.T)
    Wr = W.reshape(IN_C, HEADS, OUT_C)
    Wal = np.ascontiguousarray(np.einsum("khc,hc->kh", Wr, att_l))
    War = np.ascontiguousarray(np.einsum("khc,hc->kh", Wr, att_r))
    in_maps = []
    for m in range(NCORES):
        sl = slice(m * NB, (m + 1) * NB)
        in_maps.append({
            "xT": xT,
            "xoT": np.ascontiguousarray(x[sl].T),
            "adjbT": np.ascontiguousarray(adj[sl].T),
            "W": W,
            "Wal": Wal,
            "War": War,
            "bias": bias,
        })
    return in_maps


def _install_ntff_shim():
    # this container image lacks antenv.axon_hooks; recreate it from the boot
    # helper so run_bass_kernel_spmd's trace path can find the profile hook
    import sys, types
    if "antenv.axon_hooks" in sys.modules:
        return
    from trn_agent_boot.trn_boot import _ntff_profile_via_ctypes
    hook = _ntff_profile_via_ctypes("/opt/axon/libaxon_pjrt.so")
    mod = types.ModuleType("antenv.axon_hooks")
    mod.get_axon_ntff_profile_hook = lambda: hook
    mod.set_axon_ntff_profile_hook = lambda h: None
    sys.modules["antenv.axon_hooks"] = mod


def kernel(x, adj, W, att_l, att_r, bias):
    from concourse.bass_utils import run_bass_kernel_spmd

    if "nc" not in _compiled:
        _compiled["nc"] = build()
    nc = _compiled["nc"]
    in_maps = make_in_maps(x, adj, W, att_l, att_r, bias)
    kwargs = {}
    if TRACE:
        _install_ntff_shim()
        kwargs["trace"] = True
    res = run_bass_kernel_spmd(nc, in_maps, core_ids=list(range(NCORES)), **kwargs)
    LAST_RESULTS["exec_time_ns"] = res.exec_time_ns
    LAST_RESULTS["mean_exec_time_ns"] = res.mean_exec_time_ns
    LAST_RESULTS["res"] = res
    return np.concatenate([res.results[m]["out"] for m in range(NCORES)], axis=0)


# revision 16
# speedup vs baseline: 1.2141x; 1.2141x over previous
# DenseGATConv on 8 Trainium2 NeuronCores (Bass/Tile, SPMD over destination rows).
#
# Math: h = x@W ; el/er = head-wise <h, att> ; e_ij = leaky(el_i + er_j) ;
#       alpha = softmax_j(mask(e)) ; out_i = sum_j alpha_ij h_j + bias.
# Key identity: exp(leaky(s)) = max(exp(s), exp(0.2 s)) since exp is monotone
# and leaky(s) = max(s, 0.2 s).  With s_ij = el_i + er_j both branches are
# rank-1 outer products: exp(s) = exp(el_i) exp(er_j).  The masked unnormalized
# attention is  pm[j,i] = adj[i,j] * max(al_i*ar_j, bl_i*br_j)  which needs no
# transcendentals on the [N,N,H] tensor — just two fused DVE ops + a max.
# The denominator rides along as a ones-column in the aggregation matmul.
#
# Sharding: destination rows i split across 8 cores (512 rows each); every core
# computes the full h (it needs all source nodes j anyway); params replicated.
import numpy as np

N, IN_C, HEADS, OUT_C = 4096, 256, 4, 64
HC = HEADS * OUT_C          # 256
NCORES = 8
NB = N // NCORES            # 512 destination rows per core
JT = N // 128               # 32 source-node tiles
IT = NB // 128              # 4 row subtiles per core
C65 = OUT_C + 1             # head slice + ones column

TRACE = False               # test.py flips this to collect HW exec time
LAST_RESULTS = {}           # exec_time_ns etc. stashed here when TRACE

_compiled = {}


def _scaled_max_op():
    # Custom DVE op: out = max(in0*s0, in1*s1) with per-partition scalars.
    # Fuses the two rank-1 scalings + max of the attention kernel into one
    # instruction (per-partition-scalar tensor_scalar runs at 1x, so the
    # unfused chain costs ~3x this).  Registered dynamically into
    # concourse.dve_ops so table-gen / CoreSim / codegen all see it.
    if "op" in _compiled:
        return _compiled["op"]
    import concourse.dve_ops as dops
    from concourse.dve_ops import DveOp
    from concourse.dve_spec import Spec, Src0, Src1, C0, C1, maxx, lower
    from concourse.dve_ops import has_src1
    from concourse.dve_uop import DveOpSpec

    name = "SCALED_MAX_ANT"
    spec = Spec(
        body=maxx(Src0 * C0, Src1 * C1),
        reference=lambda in0, in1, s0, s1, imm2: np.maximum(
            in0 * s0, in1 * s1
        ).astype(np.float32),
    )
    row = max(dops._SUB_OPCODE_FOR_NAME.values()) + 1
    assert row < 0x20
    dops._SUB_OPCODE_FOR_NAME[name] = row
    shas = {}
    for ver in ("v3",):  # TRN2
        s = DveOpSpec(name=name, opcode=row, uops=lower(spec, ver=ver),
                      rd1_en=has_src1(spec))
        shas[ver] = s.sha(ver)
    op = DveOp(name, spec, subdim=False, uops_sha=shas)
    dops.OPS.append(op)
    dops.CUSTOM_DVE_SPECS[name] = spec
    _compiled["op"] = op
    return op


def _emit(ctx, tc, nc, io):
    import concourse.bass as bass
    import concourse.masks as masks
    from concourse import mybir

    dt = mybir.dt
    Alu = mybir.AluOpType
    Act = mybir.ActivationFunctionType

    xT, xoT, adjbT, W, Wal, War, bias, out = (
        io["xT"], io["xoT"], io["adjbT"], io["W"], io["Wal"], io["War"],
        io["bias"], io["out"],
    )

    big = ctx.enter_context(tc.tile_pool(name="big", bufs=1))
    tr = ctx.enter_context(tc.tile_pool(name="tr", bufs=3))
    adjpool = ctx.enter_context(tc.tile_pool(name="adjpool", bufs=2))
    ps = ctx.enter_context(tc.tile_pool(name="ps", bufs=2, space="PSUM"))
    pacc = ctx.enter_context(tc.tile_pool(name="pacc", bufs=1, space="PSUM"))

    # ---- constants / params -------------------------------------------------
    idf = big.tile([128, 128], dt.float32, tag="idf")
    masks.make_identity(nc, idf[:])
    ones_row = big.tile([1, 128], dt.float32, tag="ones_row")
    nc.vector.memset(ones_row[:], 1.0)

    bias_b = big.tile([128, HC], dt.float32, tag="bias_b")
    bias_bcast_ap = bass.AP(
        tensor=bias.tensor, offset=bias.offset, ap=[[0, 128]] + list(bias.ap)
    )
    nc.gpsimd.dma_start(out=bias_b[:], in_=bias_bcast_ap)

    wfull = []
    wb = []
    wal = []
    war = []
    for ct in range(2):
        wf = big.tile([128, HC], dt.float32, tag=f"wf{ct}")
        nc.sync.dma_start(out=wf[:], in_=W[ct * 128:(ct + 1) * 128, :])
        wfull.append(wf)
        w16 = big.tile([128, HC], dt.bfloat16, tag=f"wb{ct}")
        nc.vector.tensor_copy(w16[:], wf[:])
        wb.append(w16)
        wl = big.tile([128, HEADS], dt.float32, tag=f"wal{ct}")
        nc.sync.dma_start(out=wl[:], in_=Wal[ct * 128:(ct + 1) * 128, :])
        wal.append(wl)
        wr = big.tile([128, HEADS], dt.float32, tag=f"war{ct}")
        nc.sync.dma_start(out=wr[:], in_=War[ct * 128:(ct + 1) * 128, :])
        war.append(wr)

    # ---- xT load + bf16 cast ------------------------------------------------
    xTf = []
    xTb = []
    for ct in range(2):
        xf = big.tile([128, N], dt.float32, tag=f"xTf{ct}")
        nc.sync.dma_start(out=xf[:], in_=xT[ct * 128:(ct + 1) * 128, :])
        xTf.append(xf)
        xb = big.tile([128, N], dt.bfloat16, tag=f"xTb{ct}")
        nc.vector.tensor_copy(xb[:], xf[:])
        xTb.append(xb)
    xo = []
    for ct in range(2):
        t = big.tile([128, NB], dt.float32, tag=f"xoT{ct}")
        nc.sync.dma_start(out=t[:], in_=xoT[ct * 128:(ct + 1) * 128, :])
        xo.append(t)

    # ---- h65 (bf16 h + ones col, per j-tile) and er (f32) -------------------
    h65 = []
    er_pack = big.tile([128, JT * HEADS], dt.float32, tag="er_pack")  # [128,128]
    for nt in range(JT):
        hps = ps.tile([128, HC], dt.float32, tag="scr")
        for ct in range(2):
            nc.tensor.matmul(
                hps[:], lhsT=xTb[ct][:, nt * 128:(nt + 1) * 128], rhs=wb[ct][:],
                start=(ct == 0), stop=(ct == 1),
            )
        ht = big.tile([128, HEADS * C65], dt.bfloat16, tag=f"h65_{nt}")
        hr = ht[:].rearrange("p (h c) -> p h c", c=C65)
        hpr = hps[:].rearrange("p (h c) -> p h c", c=OUT_C)
        nc.any.tensor_copy(hr[:, :, 0:OUT_C], hpr[:, :, :])
        nc.vector.memset(hr[:, :, OUT_C], 1.0)
        h65.append(ht)

        eps = ps.tile([128, HEADS], dt.float32, tag="scr")
        for ct in range(2):
            nc.tensor.matmul(
                eps[:], lhsT=xTf[ct][:, nt * 128:(nt + 1) * 128], rhs=war[ct][:],
                start=(ct == 0), stop=(ct == 1),
            )
        nc.any.tensor_copy(er_pack[:, nt * HEADS:(nt + 1) * HEADS], eps[:])

    # exp factors for the source side (per-partition scalars, natural layout)
    ar_pack = big.tile([128, JT * HEADS], dt.float32, tag="ar_pack")
    br_pack = big.tile([128, JT * HEADS], dt.float32, tag="br_pack")
    nc.scalar.activation(ar_pack[:], er_pack[:], Act.Exp)
    nc.scalar.activation(br_pack[:], er_pack[:], Act.Exp, scale=0.2)

    # ---- elT (destination side, transposed layout) + exp + broadcast --------
    al_b = []
    bl_b = []
    for h in range(HEADS):
        # one [1, NB] matmul per head so every el row sits at partition 0
        # (engine access patterns must start at a 32-aligned partition)
        elp = ps.tile([1, NB], dt.float32, tag="scr")
        for ct in range(2):
            nc.tensor.matmul(
                elp[:], lhsT=wal[ct][:, h:h + 1], rhs=xo[ct][:],
                start=(ct == 0), stop=(ct == 1),
            )
        for scale, lst in ((1.0, al_b), (0.2, bl_b)):
            row = big.tile([1, NB], dt.float32, tag=f"albl_{h}_{scale}")
            nc.scalar.activation(row[:], elp[:], Act.Exp, scale=scale)
            bps = ps.tile([128, NB], dt.float32, tag="scr")
            nc.tensor.matmul(bps[:], lhsT=ones_row[:], rhs=row[:],
                             start=True, stop=True)
            bt = big.tile([128, NB], dt.bfloat16, tag=f"albc_{h}_{scale}")
            nc.any.tensor_copy(bt[:], bps[:])
            lst.append(bt)

    # ---- adjacency: host-pretransposed [N, NB]; load + cast bf16 (0/1) ------
    adjT = []
    for jt in range(JT):
        ai = adjpool.tile([128, NB], dt.int32, tag="adjint")
        nc.sync.dma_start(out=ai[:], in_=adjbT[jt * 128:(jt + 1) * 128, :])
        ab = big.tile([128, NB], dt.bfloat16, tag=f"adjT{jt}", name=f"adjT{jt}")
        nc.vector.tensor_copy(ab[:], ai[:])
        adjT.append(ab)

    # ---- main loop: per source tile, per head -------------------------------
    # A = (al_i*ar_j), B = (bl_i*br_j) as 4x-mode tensor_scalar; max on DVE;
    # adjacency mask multiply on GpSimd (otherwise idle) to split the load.
    smax = _scaled_max_op()
    po = [pacc.tile([C65, NB], dt.float32, name=f"o{h}", tag=f"o{h}") for h in range(HEADS)]
    for jt in range(JT):
        for h in range(HEADS):
            col = jt * HEADS + h
            q = tr.tile([128, NB], dt.bfloat16, tag="q")
            nc.vector._custom_dve(
                smax, out=q[:], in0=al_b[h][:], in1=bl_b[h][:],
                s0=ar_pack[:, col:col + 1], s1=br_pack[:, col:col + 1],
            )
            pm = tr.tile([128, NB], dt.bfloat16, tag="pm")
            if h % 2 == 0:
                nc.gpsimd.tensor_mul(pm[:], q[:], adjT[jt][:])
            else:
                nc.vector.tensor_mul(pm[:], q[:], adjT[jt][:])
            nc.tensor.matmul(
                po[h][:], lhsT=h65[jt][:, h * C65:(h + 1) * C65], rhs=pm[:],
                start=(jt == 0), stop=(jt == JT - 1),
            )

    # ---- epilogue: transpose back, normalize, bias, store -------------------
    osb = [tr.tile([C65, NB], dt.float32, name=f"osb{h}", tag=f"osb{h}") for h in range(HEADS)]
    for h in range(HEADS):
        nc.any.tensor_copy(osb[h][:], po[h][:])
    for it in range(IT):
        ot = tr.tile([128, HC], dt.float32, tag="ot")
        for h in range(HEADS):
            pt = ps.tile([128, C65], dt.float32, tag="scr")
            nc.tensor.transpose(
                pt[:], osb[h][:, it * 128:(it + 1) * 128], idf[0:C65, 0:C65]
            )
            rec = tr.tile([128, 1], dt.float32, tag="rec")
            nc.vector.reciprocal(rec[:], pt[:, OUT_C:OUT_C + 1])
            nc.vector.scalar_tensor_tensor(
                out=ot[:, h * OUT_C:(h + 1) * OUT_C], in0=pt[:, 0:OUT_C],
                scalar=rec[:], in1=bias_b[:, h * OUT_C:(h + 1) * OUT_C],
                op0=Alu.mult, op1=Alu.add,
            )
        nc.sync.dma_start(out=out[it * 128:(it + 1) * 128, :], in_=ot[:])


def build():
    from contextlib import ExitStack
    import concourse.bacc as bacc
    import concourse.tile as tile
    from concourse import mybir

    dt = mybir.dt
    nc = bacc.Bacc("TRN2", target_bir_lowering=False, debug=False,
                   num_devices=NCORES)
    io = {
        "xT": nc.dram_tensor("xT", [IN_C, N], dt.float32, kind="ExternalInput").ap(),
        "xoT": nc.dram_tensor("xoT", [IN_C, NB], dt.float32, kind="ExternalInput").ap(),
        "adjbT": nc.dram_tensor("adjbT", [N, NB], dt.int32, kind="ExternalInput").ap(),
        "W": nc.dram_tensor("W", [IN_C, HC], dt.float32, kind="ExternalInput").ap(),
        "Wal": nc.dram_tensor("Wal", [IN_C, HEADS], dt.float32, kind="ExternalInput").ap(),
        "War": nc.dram_tensor("War", [IN_C, HEADS], dt.float32, kind="ExternalInput").ap(),
        "bias": nc.dram_tensor("bias", [HC], dt.float32, kind="ExternalInput").ap(),
        "out": nc.dram_tensor("out", [NB, HC], dt.float32, kind="ExternalOutput").ap(),
    }
    with tile.TileContext(nc) as tc:
        with ExitStack() as ctx:
            _emit(ctx, tc, nc, io)
    nc.compile()
    return nc


def make_in_maps(x, adj, W, att_l, att_r, bias):
    x = np.asarray(x, np.float32)
    adj = np.ascontiguousarray(np.asarray(adj, np.int32))
    W = np.asarray(W, np.float32)
    att_l = np.asarray(att_l, np.float32)
    att_r = np.asarray(att_r, np.float32)
    bias = np.asarray(bias, np.float32)
    xT = np.ascontiguousarray(x.T)
    Wr = W.reshape(IN_C, HEADS, OUT_C)
    Wal = np.ascontiguousarray(np.einsum("khc,hc->kh", Wr, att_l))
    War = np.ascontiguousarray(np.einsum("khc,hc->kh", Wr, att_r))
    in_maps = []
    for m in range(NCORES):
        sl = slice(m * NB, (m + 1) * NB)
        in_maps.append({
            "xT": xT,
            "xoT": np.ascontiguousarray(x[sl].T),
            "adjbT": np.ascontiguousarray(adj[sl].T),
            "W": W,
            "Wal": Wal,
            "War": War,
            "bias": bias,
        })
    return in_maps


def _install_ntff_shim():
    # this container image lacks antenv.axon_hooks; recreate it from the boot
    # helper so run_bass_kernel_spmd's trace path can find the profile hook
    import sys, types
    if "antenv.axon_hooks" in sys.modules:
        return
    from trn_agent_boot.trn_boot import _ntff_profile_via_ctypes
    hook = _ntff_profile_via_ctypes("/opt/axon/libaxon_pjrt.so")
    mod = types.ModuleType("antenv.axon_hooks")
    mod.get_axon_ntff_profile_hook = lambda: hook
    mod.set_axon_ntff_profile_hook = lambda h: None
    sys.modules["antenv.axon_hooks"] = mod


def kernel(x, adj, W, att_l, att_r, bias):
    from concourse.bass_utils import run_bass_kernel_spmd

    if "nc" not in _compiled:
        _compiled["nc"] = build()
    nc = _compiled["nc"]
    in_maps = make_in_maps(x, adj, W, att_l, att_r, bias)
    kwargs = {}
    if TRACE:
        _install_ntff_shim()
        kwargs["trace"] = True
    res = run_bass_kernel_spmd(nc, in_maps, core_ids=list(range(NCORES)), **kwargs)
    LAST_RESULTS["exec_time_ns"] = res.exec_time_ns
    LAST_RESULTS["mean_exec_time_ns"] = res.mean_exec_time_ns
    LAST_RESULTS["res"] = res
    return np.concatenate([res.results[m]["out"] for m in range(NCORES)], axis=0)
